# revision 1
# baseline (speedup 1.0000x reference)
"""2-layer GCN (GCNConv x2 + log_softmax) on 8 Trainium2 NeuronCores.

Strategy (graph/data parallel per sharding hint):
  - Host: degree-sorted node partitioning across 8 cores (balances edges and
    makes ELL tiles degree-uniform), edge lists bucketed by dst owner, packed
    into per-128-dst-node-tile ELL format (slot-major), int32 indices.
    Host computes deg^-1/2 from edge_index only (graph preprocessing).
  - NEFF1: per core, hs1 = (x_shard @ W1) * dis_shard  -> [16, S] transposed.
  - host: assemble full hs1 table (layout change only).
  - NEFF2: per core, ELL gather-aggregate over its dst shard, then
    out1 = relu(agg * dis + b1);  hs2 = (out1 * dis) @ W2 -> [2, S].
  - host: assemble full hs2 table.
  - NEFF3: per core, ELL gather-aggregate (8B rows), out = log_softmax(
    agg * dis + b2) -> [S, 2]. Host unpermutes rows.

Normalization trick: norm = dis[src]*dis[dst] factorizes, so we pre-scale the
message table by dis (producer side) and post-scale the aggregate by dis
(consumer side); no per-edge scaling needed.
"""

import math
import sys

import numpy as np

sys.path.insert(0, "/opt/trn_rl_repo")

from contextlib import ExitStack

import concourse.bacc as bacc
import concourse.tile as tile
from concourse import bass, mybir
from concourse.bass_utils import run_bass_kernel_spmd
from concourse.masks import make_identity

N_NODES = 100000
N_CORES = 8
P = 128
SHARD = 12544  # 98 * 128, padded per-core shard size
N_TILES = SHARD // P  # 98
F_IN, HID, OUT = 128, 16, 2
TABLE_ROWS = N_CORES * SHARD  # 100352
PAD_ROW = 12500  # core 0's first zero pad slot -> global row 12500 is zeros

_CACHE = {}


# ----------------------------------------------------------------- host prep
def _preprocess(edge_index):
    src = edge_index[0].astype(np.int64)
    dst = edge_index[1].astype(np.int64)
    loops = np.arange(N_NODES, dtype=np.int64)
    src = np.concatenate([src, loops])
    dst = np.concatenate([dst, loops])

    deg = np.bincount(dst, minlength=N_NODES).astype(np.float64)
    dis = (1.0 / np.sqrt(np.maximum(deg, 1.0))).astype(np.float32)

    # degree-sorted deal: rank r -> core r%8, slot r//8
    ranked = np.argsort(-deg, kind="stable")  # node ids by degree desc
    rank_of = np.empty(N_NODES, dtype=np.int64)
    rank_of[ranked] = np.arange(N_NODES)
    core_of = rank_of % N_CORES
    slot_of = rank_of // N_CORES
    # global permuted table row for node n
    grow_of = (core_of * SHARD + slot_of).astype(np.int64)

    e_core = core_of[dst]
    e_slot = slot_of[dst]
    e_gsrc = grow_of[src].astype(np.int32)

    # per (core, tile, row) counts to find D[t] = max over cores+rows
    e_tile = e_slot // P
    e_row = e_slot % P
    # counts[core, tile, row]
    flat = (e_core * SHARD + e_slot).astype(np.int64)
    cnt = np.bincount(flat, minlength=N_CORES * SHARD).reshape(N_CORES, N_TILES, P)
    D = cnt.max(axis=(0, 2)).astype(np.int64)  # [N_TILES]
    D = np.maximum(D, 1)
    doff = np.concatenate([[0], np.cumsum(D)])
    sum_d = int(doff[-1])

    # pack ELL: ell[core][128, sum_d], tile t occupies cols doff[t]:doff[t]+D[t]
    ell = np.full((N_CORES, P, sum_d), PAD_ROW, dtype=np.int32)
    order = np.lexsort((e_slot, e_core))
    oc, ot, orow, ogs = e_core[order], e_tile[order], e_row[order], e_gsrc[order]
    # j-index within (core,slot) groups: order is sorted by (core, slot)
    okey = (oc * SHARD + ot * P + orow)
    uniq, first_idx = np.unique(okey, return_index=True)
    j_idx = np.arange(len(okey)) - np.repeat(first_idx, np.diff(np.concatenate([first_idx, [len(okey)]])))
    ell[oc, orow, doff[ot] + j_idx] = ogs

    # dis in the two layouts the kernels use
    dis_shard = np.zeros((N_CORES, SHARD), dtype=np.float32)
    for c in range(N_CORES):
        ids = ranked[c::N_CORES]
        dis_shard[c, : len(ids)] = dis[ids]
    dis2d = dis_shard.reshape(N_CORES, N_TILES, P).transpose(0, 2, 1)  # [C,128,98]

    return {
        "ranked": ranked,
        "ell": ell,
        "D": [int(d) for d in D],
        "doff": doff,
        "sum_d": sum_d,
        "dis_shard": dis_shard,
        "dis2d": np.ascontiguousarray(dis2d),
    }


# ------------------------------------------------------------- NEFF builders
def _build_neff1():
    """x_shard [SHARD,128] @ W1 [128,16] * dis -> hs1T [16, SHARD]"""
    nc = bacc.Bacc(None, target_bir_lowering=False, debug=True)
    with tile.TileContext(nc) as tc:
        with ExitStack() as ctx:
            dram = ctx.enter_context(tc.tile_pool(name="dram", bufs=1, space="DRAM"))
            x_d = dram.tile([SHARD, F_IN], mybir.dt.float32, kind="ExternalInput", name="x", uniquify=False)
            w1_d = dram.tile([F_IN, HID], mybir.dt.float32, kind="ExternalInput", name="w1", uniquify=False)
            dis_d = dram.tile([P, N_TILES], mybir.dt.float32, kind="ExternalInput", name="dis2d", uniquify=False)
            out_d = dram.tile([HID, SHARD], mybir.dt.float32, kind="ExternalOutput", name="hs1T", uniquify=False)

            sb = ctx.enter_context(tc.tile_pool(name="sb", bufs=3))
            sb1 = ctx.enter_context(tc.tile_pool(name="sb1", bufs=1))
            ps = ctx.enter_context(tc.tile_pool(name="ps", bufs=3, space="PSUM"))

            ident = sb1.tile([P, P], mybir.dt.float32)
            make_identity(nc, ident[:])
            w1_sb = sb1.tile([F_IN, HID], mybir.dt.float32)
            nc.sync.dma_start(out=w1_sb[:], in_=w1_d[:])
            dis_sb = sb1.tile([P, N_TILES], mybir.dt.float32)
            nc.sync.dma_start(out=dis_sb[:], in_=dis_d[:])

            for t in range(N_TILES):
                xt = sb.tile([P, F_IN], mybir.dt.float32, tag="xt")
                nc.sync.dma_start(out=xt[:], in_=x_d[t * P : (t + 1) * P, :])
                xs = sb.tile([P, F_IN], mybir.dt.float32, tag="xs")
                nc.vector.tensor_tensor(
                    out=xs[:],
                    in0=xt[:],
                    in1=dis_sb[:, t : t + 1].to_broadcast([P, F_IN]),
                    op=mybir.AluOpType.mult,
                )
                xT_ps = ps.tile([F_IN, P], mybir.dt.float32, tag="xT")
                nc.tensor.transpose(out=xT_ps[:], in_=xs[:], identity=ident[:])
                xT_sb = sb.tile([F_IN, P], mybir.dt.float32, tag="xTsb")
                nc.vector.tensor_copy(out=xT_sb[:], in_=xT_ps[:])
                hT_ps = ps.tile([HID, P], mybir.dt.float32, tag="hT")
                nc.tensor.matmul(out=hT_ps[:], lhsT=w1_sb[:], rhs=xT_sb[:], start=True, stop=True)
                hsT_sb = sb.tile([HID, P], mybir.dt.float32, tag="hsT")
                nc.vector.tensor_copy(out=hsT_sb[:], in_=hT_ps[:])
                nc.sync.dma_start(out=out_d[:, t * P : (t + 1) * P], in_=hsT_sb[:])
    nc.compile()
    return nc


def _build_agg_neff(D, doff, sum_d, feat, layer):
    """Shared builder for NEFF2 (layer=1, feat=16) and NEFF3 (layer=2, feat=2)."""
    nc = bacc.Bacc(None, target_bir_lowering=False, debug=True)
    fp32 = mybir.dt.float32
    # lanes of `feat` wide accumulate buffer
    WIDE = 128 if layer == 1 else 32
    LANES = WIDE // feat  # 8 or 16
    with tile.TileContext(nc) as tc:
        with ExitStack() as ctx:
            dram = ctx.enter_context(tc.tile_pool(name="dram", bufs=1, space="DRAM"))
            table_d = dram.tile([TABLE_ROWS, feat], fp32, kind="ExternalInput", name="table", uniquify=False)
            ell_d = dram.tile([P, sum_d], mybir.dt.int32, kind="ExternalInput", name="ell", uniquify=False)
            dis_d = dram.tile([P, N_TILES], fp32, kind="ExternalInput", name="dis2d", uniquify=False)
            if layer == 1:
                b_d = dram.tile([P, HID], fp32, kind="ExternalInput", name="b1", uniquify=False)
                w2_d = dram.tile([HID, OUT], fp32, kind="ExternalInput", name="w2", uniquify=False)
                out_d = dram.tile([OUT, SHARD], fp32, kind="ExternalOutput", name="hs2T", uniquify=False)
            else:
                b_d = dram.tile([P, OUT], fp32, kind="ExternalInput", name="b2", uniquify=False)
                out_d = dram.tile([SHARD, OUT], fp32, kind="ExternalOutput", name="out", uniquify=False)

            sb = ctx.enter_context(tc.tile_pool(name="sb", bufs=3))
            sb1 = ctx.enter_context(tc.tile_pool(name="sb1", bufs=1))
            ps = ctx.enter_context(tc.tile_pool(name="ps", bufs=3, space="PSUM"))

            dis_sb = sb1.tile([P, N_TILES], fp32)
            nc.sync.dma_start(out=dis_sb[:], in_=dis_d[:])
            b_sb = sb1.tile([P, feat if layer == 2 else HID], fp32)
            nc.sync.dma_start(out=b_sb[:], in_=b_d[:])
            if layer == 1:
                ident = sb1.tile([P, P], fp32)
                make_identity(nc, ident[:])
                w2_sb = sb1.tile([HID, OUT], fp32)
                nc.sync.dma_start(out=w2_sb[:], in_=w2_d[:])

            for t in range(N_TILES):
                d = D[t]
                o = int(doff[t])
                idx = sb.tile([P, max(D)], mybir.dt.int32, tag="idx")
                nc.sync.dma_start(out=idx[:, :d], in_=ell_d[:, o : o + d])
                acc = sb.tile([P, WIDE], fp32, tag="acc")
                nc.vector.memset(acc[:], 0.0)
                msgw = sb.tile([P, WIDE], fp32, tag="msgw")
                n_groups = math.ceil(d / LANES)
                for g in range(n_groups):
                    lanes = min(LANES, d - g * LANES)
                    if lanes < LANES:
                        nc.vector.memset(msgw[:, lanes * feat :], 0.0)
                    for j in range(lanes):
                        sl = g * LANES + j
                        nc.gpsimd.indirect_dma_start(
                            out=msgw[:, j * feat : (j + 1) * feat],
                            out_offset=None,
                            in_=table_d[:],
                            in_offset=bass.IndirectOffsetOnAxis(ap=idx[:, sl : sl + 1], axis=0),
                        )
                    nc.vector.tensor_tensor(
                        out=acc[:], in0=acc[:], in1=msgw[:], op=mybir.AluOpType.add
                    )
                # fold lanes
                w = WIDE
                while w > feat:
                    w //= 2
                    nc.vector.tensor_tensor(
                        out=acc[:, :w], in0=acc[:, :w], in1=acc[:, w : 2 * w], op=mybir.AluOpType.add
                    )
                agg = acc[:, :feat]
                disb = dis_sb[:, t : t + 1].to_broadcast([P, feat])
                scaled = sb.tile([P, feat], fp32, tag="scaled")
                nc.vector.tensor_tensor(out=scaled[:], in0=agg, in1=disb, op=mybir.AluOpType.mult)
                biased = sb.tile([P, feat], fp32, tag="biased")
                nc.vector.tensor_tensor(
                    out=biased[:], in0=scaled[:], in1=b_sb[:, :feat], op=mybir.AluOpType.add
                )
                if layer == 1:
                    r = sb.tile([P, HID], fp32, tag="relu")
                    nc.scalar.activation(out=r[:], in_=biased[:], func=mybir.ActivationFunctionType.Relu)
                    r2 = sb.tile([P, HID], fp32, tag="r2")
                    nc.vector.tensor_tensor(
                        out=r2[:], in0=r[:], in1=dis_sb[:, t : t + 1].to_broadcast([P, HID]), op=mybir.AluOpType.mult
                    )
                    rT_ps = ps.tile([HID, P], fp32, tag="rT")
                    nc.tensor.transpose(out=rT_ps[:], in_=r2[:], identity=ident[:])
                    rT_sb = sb.tile([HID, P], fp32, tag="rTsb")
                    nc.vector.tensor_copy(out=rT_sb[:], in_=rT_ps[:])
                    h2T_ps = ps.tile([OUT, P], fp32, tag="h2T")
                    nc.tensor.matmul(out=h2T_ps[:], lhsT=w2_sb[:], rhs=rT_sb[:], start=True, stop=True)
                    h2T_sb = sb.tile([OUT, P], fp32, tag="h2Tsb")
                    nc.vector.tensor_copy(out=h2T_sb[:], in_=h2T_ps[:])
                    nc.sync.dma_start(out=out_d[:, t * P : (t + 1) * P], in_=h2T_sb[:])
                else:
                    # log_softmax over the 2 columns
                    mx = sb.tile([P, 1], fp32, tag="mx")
                    nc.vector.tensor_reduce(out=mx[:], in_=biased[:], axis=mybir.AxisListType.X, op=mybir.AluOpType.max)
                    sh = sb.tile([P, OUT], fp32, tag="sh")
                    nc.vector.tensor_tensor(
                        out=sh[:], in0=biased[:], in1=mx[:].to_broadcast([P, OUT]), op=mybir.AluOpType.subtract
                    )
                    ex = sb.tile([P, OUT], fp32, tag="ex")
                    nc.scalar.activation(out=ex[:], in_=sh[:], func=mybir.ActivationFunctionType.Exp)
                    sm = sb.tile([P, 1], fp32, tag="sm")
                    nc.vector.tensor_reduce(out=sm[:], in_=ex[:], axis=mybir.AxisListType.X, op=mybir.AluOpType.add)
                    ls = sb.tile([P, 1], fp32, tag="ls")
                    nc.scalar.activation(out=ls[:], in_=sm[:], func=mybir.ActivationFunctionType.Ln)
                    res = sb.tile([P, OUT], fp32, tag="res")
                    nc.vector.tensor_tensor(
                        out=res[:], in0=sh[:], in1=ls[:].to_broadcast([P, OUT]), op=mybir.AluOpType.subtract
                    )
                    nc.sync.dma_start(out=out_d[t * P : (t + 1) * P, :], in_=res[:])
    nc.compile()
    return nc


# ------------------------------------------------------------------- driver
def kernel(x, edge_index, W1, b1, W2, b2, _profile=False):
    x = np.asarray(x, dtype=np.float32)
    W1 = np.asarray(W1, dtype=np.float32)
    b1 = np.asarray(b1, dtype=np.float32)
    W2 = np.asarray(W2, dtype=np.float32)
    b2 = np.asarray(b2, dtype=np.float32)
    pp = _preprocess(np.asarray(edge_index))
    ranked, ell, D, doff, sum_d = pp["ranked"], pp["ell"], pp["D"], pp["doff"], pp["sum_d"]

    key = ("neffs", tuple(D))
    if key not in _CACHE:
        _CACHE[key] = (
            _build_neff1(),
            _build_agg_neff(D, doff, sum_d, HID, layer=1),
            _build_agg_neff(D, doff, sum_d, OUT, layer=2),
        )
    nc1, nc2, nc3 = _CACHE[key]
    cores = list(range(N_CORES))
    prof = []

    # NEFF1
    in1 = []
    for c in cores:
        ids = ranked[c::N_CORES]
        xs = np.zeros((SHARD, F_IN), dtype=np.float32)
        xs[: len(ids)] = x[ids]
        in1.append({"x": xs, "w1": W1, "dis2d": pp["dis2d"][c]})
    r1 = run_bass_kernel_spmd(nc1, in1, cores, trace=False)
    prof.append(r1)
    hs1 = np.concatenate([r1.results[c]["hs1T"].T for c in cores], axis=0)
    hs1 = np.ascontiguousarray(hs1)  # [TABLE_ROWS, 16]

    # NEFF2
    in2 = [
        {"table": hs1, "ell": ell[c], "dis2d": pp["dis2d"][c], "b1": np.tile(b1[None, :], (128, 1)), "w2": W2}
        for c in cores
    ]
    r2 = run_bass_kernel_spmd(nc2, in2, cores, trace=False)
    prof.append(r2)
    hs2 = np.concatenate([r2.results[c]["hs2T"].T for c in cores], axis=0)
    hs2 = np.ascontiguousarray(hs2)  # [TABLE_ROWS, 2]

    # NEFF3
    in3 = [
        {"table": hs2, "ell": ell[c], "dis2d": pp["dis2d"][c], "b2": np.tile(b2[None, :], (128, 1))}
        for c in cores
    ]
    r3 = run_bass_kernel_spmd(nc3, in3, cores, trace=False)
    prof.append(r3)
    kernel._last_inmaps = (in1, in2, in3)
    kernel._last_ncs = (nc1, nc2, nc3)

    out = np.empty((N_NODES, OUT), dtype=np.float32)
    for c in cores:
        ids = ranked[c::N_CORES]
        out[ids] = r3.results[c]["out"][: len(ids)]
    if _profile:
        kernel._last_profile = prof
    return out



# revision 4
# speedup vs baseline: 1.2692x; 1.2692x over previous
"""2-layer GCN (GCNConv x2 + log_softmax) on 8 Trainium2 NeuronCores.

Strategy (graph/data parallel per sharding hint):
  - Host: degree-sorted node partitioning across 8 cores, edges bucketed by
    dst owner into per-128-dst-node-tile ELL format. The message table is
    packed 4 nodes per 256-byte row ([25088, 64] fp32) so the aggregation
    uses ONE bulk SWDGE dma_gather per dst tile (int16 indices cover the
    25088 packed rows), instead of per-slot indirect DMAs.
  - A gathered 256B row holds 4 nodes' 16-float feature blocks; a host-built
    class id (node%4) is expanded on-device into a {0,1} mask [128, 4D, 16]
    that selects the right sub-block, and a strided tensor_reduce sums the
    masked messages per dst row. norm = dis[src]*dis[dst] factorizes:
    dis[src] is pre-folded into the tables, dis[dst] applied post-aggregate.
  - NEFF1: hs1T = W1.T @ (x*dis).T per core -> host assembles table1.
  - NEFF2: gather-aggregate table1 -> r = relu(agg*dis + b1) * dis
    (the trailing *dis pre-folds layer-2's src scaling) -> table2.
  - NEFF3: gather-aggregate table2 -> out = log_softmax((agg*dis)@W2 + b2).
"""

import sys

import numpy as np

sys.path.insert(0, "/opt/trn_rl_repo")

from contextlib import ExitStack

import concourse.bacc as bacc
import concourse.tile as tile
from concourse import bass, mybir
from concourse.bass_utils import run_bass_kernel_spmd
from concourse.masks import make_identity

N_NODES = 100000
N_CORES = 8
P = 128
SHARD = 12544  # 98 * 128, padded per-core shard size
N_TILES = SHARD // P  # 98
F_IN, HID, OUT = 128, 16, 2
TABLE_ROWS = N_CORES * SHARD  # 100352
PROWS = TABLE_ROWS // 4  # 25088 packed table rows (4 nodes x 16 f32 = 256B)
EXP_CHUNK = 7  # tiles per wmask-expansion chunk (98 = 14*7)

_CACHE = {}


# ----------------------------------------------------------------- host prep
def _preprocess(edge_index):
    src = edge_index[0].astype(np.int64)
    dst = edge_index[1].astype(np.int64)
    loops = np.arange(N_NODES, dtype=np.int64)
    src = np.concatenate([src, loops])
    dst = np.concatenate([dst, loops])

    deg = np.bincount(dst, minlength=N_NODES).astype(np.float64)
    dis = (1.0 / np.sqrt(np.maximum(deg, 1.0))).astype(np.float32)

    # degree-sorted deal: rank r -> core r%8, slot r//8 (tiles are tight
    # degree bands of 1024 ranks -> minimal ELL padding)
    ranked = np.argsort(-deg, kind="stable")
    rank_of = np.empty(N_NODES, dtype=np.int64)
    rank_of[ranked] = np.arange(N_NODES)
    core_of = rank_of % N_CORES
    slot_of = rank_of // N_CORES
    grow_of = (core_of * SHARD + slot_of).astype(np.int64)  # global table slot
    prow_of = (grow_of // 4).astype(np.int16)  # packed row
    pcls_of = (grow_of % 4).astype(np.float32)  # class within packed row

    e_core = core_of[dst]
    e_slot = slot_of[dst]
    e_tile = e_slot // P
    e_row = e_slot % P

    # per (core, tile, row) counts -> D[t] = max over cores+rows
    flat = (e_core * SHARD + e_slot).astype(np.int64)
    cnt = np.bincount(flat, minlength=N_CORES * SHARD).reshape(N_CORES, N_TILES, P)
    D = np.maximum(cnt.max(axis=(0, 2)), 1).astype(np.int64)  # [N_TILES]
    doff = np.concatenate([[0], np.cumsum(D)])
    sumd = int(doff[-1])

    # pack ELL [core][128, sumd]: tile t occupies cols doff[t]:doff[t]+D[t].
    # pads: prow=0 with class=4.0 (no class matches -> masked to zero)
    ellp = np.zeros((N_CORES, P, sumd), dtype=np.int16)
    ellc = np.full((N_CORES, P, sumd), 4.0, dtype=np.float32)
    order = np.lexsort((e_slot, e_core))
    oc, ot, orow = e_core[order], e_tile[order], e_row[order]
    osrc = src[order]
    okey = oc * SHARD + ot * P + orow
    uniq, first_idx = np.unique(okey, return_index=True)
    j_idx = np.arange(len(okey)) - np.repeat(
        first_idx, np.diff(np.concatenate([first_idx, [len(okey)]]))
    )
    col = doff[ot] + j_idx
    ellp[oc, orow, col] = prow_of[osrc]
    ellc[oc, orow, col] = pcls_of[osrc]

    # dma_gather idx strip [16, 8*sumd] per core: position i=(j*128+p) of tile
    # t lives at [(i%16), 8*doff[t] + i//16]; device replicates to 128 parts.
    jj = np.arange(P * int(D.max()))
    idx16 = np.zeros((N_CORES, 16, 8 * sumd), dtype=np.int16)
    for t in range(N_TILES):
        d = int(D[t])
        i = jj[: P * d]
        p = i % P
        j = i // P
        # vals[i] = ellp[:, p, doff+j] -> [C, 128d]
        vals = ellp[:, p, int(doff[t]) + j]
        strip = vals.reshape(N_CORES, 8 * d, 16).transpose(0, 2, 1)
        idx16[:, :, 8 * int(doff[t]) : 8 * (int(doff[t]) + d)] = strip

    # dis in [C, 128, 98] layout (dst scaling per tile column)
    dis_shard = np.zeros((N_CORES, SHARD), dtype=np.float32)
    for c in range(N_CORES):
        ids = ranked[c::N_CORES]
        dis_shard[c, : len(ids)] = dis[ids]
    dis2d = np.ascontiguousarray(
        dis_shard.reshape(N_CORES, N_TILES, P).transpose(0, 2, 1)
    )

    return {
        "ranked": ranked,
        "D": [int(x) for x in D],
        "doff": [int(x) for x in doff],
        "sumd": sumd,
        "idx16": idx16,
        "wcls": ellc,
        "dis2d": dis2d,
    }


# ------------------------------------------------------------- NEFF builders
def _build_neff1():
    """x_shard [SHARD,128] @ W1 [128,16] * dis -> hs1T [16, SHARD]"""
    nc = bacc.Bacc(None, target_bir_lowering=False, debug=True)
    with tile.TileContext(nc) as tc:
        with ExitStack() as ctx:
            dram = ctx.enter_context(tc.tile_pool(name="dram", bufs=1, space="DRAM"))
            x_d = dram.tile([SHARD, F_IN], mybir.dt.float32, kind="ExternalInput", name="x", uniquify=False)
            w1_d = dram.tile([F_IN, HID], mybir.dt.float32, kind="ExternalInput", name="w1", uniquify=False)
            dis_d = dram.tile([P, N_TILES], mybir.dt.float32, kind="ExternalInput", name="dis2d", uniquify=False)
            out_d = dram.tile([HID, SHARD], mybir.dt.float32, kind="ExternalOutput", name="hs1T", uniquify=False)

            sb = ctx.enter_context(tc.tile_pool(name="sb", bufs=3))
            sb1 = ctx.enter_context(tc.tile_pool(name="sb1", bufs=1))
            ps = ctx.enter_context(tc.tile_pool(name="ps", bufs=3, space="PSUM"))

            ident = sb1.tile([P, P], mybir.dt.float32)
            make_identity(nc, ident[:])
            w1_sb = sb1.tile([F_IN, HID], mybir.dt.float32)
            nc.sync.dma_start(out=w1_sb[:], in_=w1_d[:])
            dis_sb = sb1.tile([P, N_TILES], mybir.dt.float32)
            nc.sync.dma_start(out=dis_sb[:], in_=dis_d[:])

            for t in range(N_TILES):
                xt = sb.tile([P, F_IN], mybir.dt.float32, tag="xt")
                nc.sync.dma_start(out=xt[:], in_=x_d[t * P : (t + 1) * P, :])
                xs = sb.tile([P, F_IN], mybir.dt.float32, tag="xs")
                nc.vector.tensor_tensor(
                    out=xs[:],
                    in0=xt[:],
                    in1=dis_sb[:, t : t + 1].to_broadcast([P, F_IN]),
                    op=mybir.AluOpType.mult,
                )
                xT_ps = ps.tile([F_IN, P], mybir.dt.float32, tag="xT")
                nc.tensor.transpose(out=xT_ps[:], in_=xs[:], identity=ident[:])
                xT_sb = sb.tile([F_IN, P], mybir.dt.float32, tag="xTsb")
                nc.vector.tensor_copy(out=xT_sb[:], in_=xT_ps[:])
                hT_ps = ps.tile([HID, P], mybir.dt.float32, tag="hT")
                nc.tensor.matmul(out=hT_ps[:], lhsT=w1_sb[:], rhs=xT_sb[:], start=True, stop=True)
                hsT_sb = sb.tile([HID, P], mybir.dt.float32, tag="hsT")
                nc.vector.tensor_copy(out=hsT_sb[:], in_=hT_ps[:])
                nc.sync.dma_start(out=out_d[:, t * P : (t + 1) * P], in_=hsT_sb[:])
    nc.compile()
    return nc


def _build_agg_neff(D, doff, sumd, layer):
    """Gather-aggregate over the packed table. layer=1: -> r*dis packed rows.
    layer=2: -> log_softmax((agg*dis)@W2 + b2) rows."""
    nc = bacc.Bacc(None, target_bir_lowering=False, debug=True)
    fp32, i16 = mybir.dt.float32, mybir.dt.int16
    dmax = max(D)
    with tile.TileContext(nc) as tc:
        with ExitStack() as ctx:
            dram = ctx.enter_context(tc.tile_pool(name="dram", bufs=1, space="DRAM"))
            table_d = dram.tile([PROWS, 64], fp32, kind="ExternalInput", name="table", uniquify=False)
            idx_d = dram.tile([16, 8 * sumd], i16, kind="ExternalInput", name="idx16", uniquify=False)
            cls_d = dram.tile([P, sumd], fp32, kind="ExternalInput", name="wcls", uniquify=False)
            dis_d = dram.tile([P, N_TILES], fp32, kind="ExternalInput", name="dis2d", uniquify=False)
            if layer == 1:
                b1_d = dram.tile([P, HID], fp32, kind="ExternalInput", name="b1row", uniquify=False)
                out_d = dram.tile([SHARD // 4, 64], fp32, kind="ExternalOutput", name="rpk", uniquify=False)
            else:
                w2_d = dram.tile([P, OUT * HID], fp32, kind="ExternalInput", name="w2cols", uniquify=False)
                b2_d = dram.tile([P, OUT], fp32, kind="ExternalInput", name="b2row", uniquify=False)
                out_d = dram.tile([SHARD, OUT], fp32, kind="ExternalOutput", name="out", uniquify=False)

            sb1 = ctx.enter_context(tc.tile_pool(name="sb1", bufs=1))
            sb = ctx.enter_context(tc.tile_pool(name="sb", bufs=3))
            sbg = ctx.enter_context(tc.tile_pool(name="sbg", bufs=2))
            sbw = ctx.enter_context(tc.tile_pool(name="sbw", bufs=2))
            sbp = ctx.enter_context(tc.tile_pool(name="sbp", bufs=1))

            idx_sb = sb1.tile([P, 8 * sumd], i16)
            nc.sync.dma_start(out=idx_sb[0:16, :], in_=idx_d[:])
            for g in range(1, 8):
                nc.sync.dma_start(
                    out=idx_sb[16 * g : 16 * (g + 1), :], in_=idx_sb[0:16, :]
                )
            cls_sb = sb1.tile([P, sumd], fp32)
            nc.sync.dma_start(out=cls_sb[:], in_=cls_d[:])
            dis_sb = sb1.tile([P, N_TILES], fp32)
            nc.sync.dma_start(out=dis_sb[:], in_=dis_d[:])
            if layer == 1:
                b1_sb = sb1.tile([P, HID], fp32)
                nc.sync.dma_start(out=b1_sb[:], in_=b1_d[:])
            else:
                w2_sb = sb1.tile([P, OUT * HID], fp32)
                nc.sync.dma_start(out=w2_sb[:], in_=w2_d[:])
                b2_sb = sb1.tile([P, OUT], fp32)
                nc.sync.dma_start(out=b2_sb[:], in_=b2_d[:])

            # chunked one-hot class-mask expansion: wm[p, 4s+k] = (wcls[p,s]==k)
            n_chunks = (N_TILES + EXP_CHUNK - 1) // EXP_CHUNK
            chunk_of = [t // EXP_CHUNK for t in range(N_TILES)]
            chunk_lo = [doff[c * EXP_CHUNK] for c in range(n_chunks)]
            chunk_sz = [
                doff[min((c + 1) * EXP_CHUNK, N_TILES)] - doff[c * EXP_CHUNK]
                for c in range(n_chunks)
            ]
            wm_max = max(chunk_sz)
            wm_tiles = {}

            def expand_chunk(c):
                lo, csz = chunk_lo[c], chunk_sz[c]
                wm = sbw.tile([P, 4 * wm_max], fp32, tag="wm")
                wm3 = wm[:, : 4 * csz].rearrange("p (s k) -> p s k", k=4)
                for k in range(4):
                    nc.vector.tensor_scalar(
                        out=wm3[:, :, k : k + 1],
                        in0=cls_sb[:, lo : lo + csz].unsqueeze(2),
                        scalar1=float(k),
                        scalar2=None,
                        op0=mybir.AluOpType.is_equal,
                    )
                wm_tiles[c] = wm

            for t in range(N_TILES):
                d, o = D[t], doff[t]
                c = chunk_of[t]
                if t % EXP_CHUNK == 0:
                    expand_chunk(c)
                wm = wm_tiles[c]
                so = o - chunk_lo[c]  # slot offset within chunk

                buf = sbg.tile([P, dmax * 64], fp32, tag="buf")
                nc.gpsimd.dma_gather(
                    buf[:, : d * 64].rearrange("p (s f) -> p s f", f=64),
                    table_d[:],
                    idx_sb[:, 8 * o : 8 * (o + d)],
                    P * d,
                    P * d,
                    64,
                    single_packet=False,
                )
                prod = sbp.tile([P, 4 * dmax * HID], fp32, tag="prod")
                pv = prod[:, : 4 * d * HID].rearrange("p (s f) -> p s f", f=HID)
                nc.vector.tensor_tensor(
                    out=pv,
                    in0=buf[:, : d * 64].rearrange("p (s f) -> p s f", f=HID),
                    in1=wm[:, 4 * so : 4 * (so + d)]
                    .unsqueeze(2)
                    .to_broadcast([P, 4 * d, HID]),
                    op=mybir.AluOpType.mult,
                )
                agg = sb.tile([P, HID], fp32, tag="agg")
                nc.vector.tensor_reduce(
                    out=agg[:],
                    in_=pv.rearrange("p s f -> p f s"),
                    axis=mybir.AxisListType.X,
                    op=mybir.AluOpType.add,
                )
                discol = dis_sb[:, t : t + 1]
                if layer == 1:
                    y = sb.tile([P, HID], fp32, tag="y")
                    nc.vector.tensor_tensor(
                        out=y[:],
                        in0=agg[:],
                        in1=discol.to_broadcast([P, HID]),
                        op=mybir.AluOpType.mult,
                    )
                    y2 = sb.tile([P, HID], fp32, tag="y2")
                    nc.vector.tensor_tensor(
                        out=y2[:], in0=y[:], in1=b1_sb[:], op=mybir.AluOpType.add
                    )
                    # r*dis = relu(y2*dis) since dis>0
                    r2 = sb.tile([P, HID], fp32, tag="r2")
                    nc.scalar.activation(
                        out=r2[:],
                        in_=y2[:],
                        func=mybir.ActivationFunctionType.Relu,
                        scale=discol,
                    )
                    nc.sync.dma_start(
                        out=out_d[32 * t : 32 * (t + 1), :].rearrange(
                            "a (b f) -> (a b) f", b=4, f=HID
                        ),
                        in_=r2[:],
                    )
                else:
                    z = sb.tile([P, HID], fp32, tag="z")
                    nc.vector.tensor_tensor(
                        out=z[:],
                        in0=agg[:],
                        in1=discol.to_broadcast([P, HID]),
                        op=mybir.AluOpType.mult,
                    )
                    o2 = sb.tile([P, OUT], fp32, tag="o2")
                    zc = sb.tile([P, OUT * HID], fp32, tag="zc")
                    nc.vector.tensor_tensor(
                        out=zc[:].rearrange("p (c f) -> p c f", f=HID),
                        in0=z[:].unsqueeze(1).to_broadcast([P, OUT, HID]),
                        in1=w2_sb[:].rearrange("p (c f) -> p c f", f=HID),
                        op=mybir.AluOpType.mult,
                    )
                    nc.vector.tensor_reduce(
                        out=o2[:],
                        in_=zc[:].rearrange("p (c f) -> p c f", f=HID),
                        axis=mybir.AxisListType.X,
                        op=mybir.AluOpType.add,
                    )
                    o2b = sb.tile([P, OUT], fp32, tag="o2b")
                    nc.vector.tensor_tensor(
                        out=o2b[:], in0=o2[:], in1=b2_sb[:], op=mybir.AluOpType.add
                    )
                    mx = sb.tile([P, 1], fp32, tag="mx")
                    nc.vector.tensor_reduce(
                        out=mx[:], in_=o2b[:], axis=mybir.AxisListType.X, op=mybir.AluOpType.max
                    )
                    sh = sb.tile([P, OUT], fp32, tag="sh")
                    nc.vector.tensor_tensor(
                        out=sh[:], in0=o2b[:], in1=mx[:].to_broadcast([P, OUT]), op=mybir.AluOpType.subtract
                    )
                    ex = sb.tile([P, OUT], fp32, tag="ex")
                    nc.scalar.activation(
                        out=ex[:], in_=sh[:], func=mybir.ActivationFunctionType.Exp
                    )
                    sm = sb.tile([P, 1], fp32, tag="sm")
                    nc.vector.tensor_reduce(
                        out=sm[:], in_=ex[:], axis=mybir.AxisListType.X, op=mybir.AluOpType.add
                    )
                    ls = sb.tile([P, 1], fp32, tag="ls")
                    nc.scalar.activation(
                        out=ls[:], in_=sm[:], func=mybir.ActivationFunctionType.Ln
                    )
                    res = sb.tile([P, OUT], fp32, tag="res")
                    nc.vector.tensor_tensor(
                        out=res[:], in0=sh[:], in1=ls[:].to_broadcast([P, OUT]), op=mybir.AluOpType.subtract
                    )
                    nc.sync.dma_start(out=out_d[t * P : (t + 1) * P, :], in_=res[:])
    nc.compile()
    return nc


# ------------------------------------------------------------------- driver
def kernel(x, edge_index, W1, b1, W2, b2, _profile=False):
    x = np.asarray(x, dtype=np.float32)
    W1 = np.asarray(W1, dtype=np.float32)
    b1 = np.asarray(b1, dtype=np.float32)
    W2 = np.asarray(W2, dtype=np.float32)
    b2 = np.asarray(b2, dtype=np.float32)
    pp = _preprocess(np.asarray(edge_index))
    ranked, D, doff, sumd = pp["ranked"], pp["D"], pp["doff"], pp["sumd"]

    key = ("neffs", tuple(D))
    if key not in _CACHE:
        _CACHE[key] = (
            _build_neff1(),
            _build_agg_neff(D, doff, sumd, layer=1),
            _build_agg_neff(D, doff, sumd, layer=2),
        )
    nc1, nc2, nc3 = _CACHE[key]
    cores = list(range(N_CORES))

    # NEFF1
    in1 = []
    for c in cores:
        ids = ranked[c::N_CORES]
        xs = np.zeros((SHARD, F_IN), dtype=np.float32)
        xs[: len(ids)] = x[ids]
        in1.append({"x": xs, "w1": W1, "dis2d": pp["dis2d"][c]})
    r1 = run_bass_kernel_spmd(nc1, in1, cores, trace=False)
    table1 = np.ascontiguousarray(
        np.concatenate([r1.results[c]["hs1T"].T for c in cores], axis=0)
    ).reshape(PROWS, 64)

    # NEFF2
    b1row = np.tile(b1[None, :], (P, 1)).astype(np.float32)
    in2 = [
        {
            "table": table1,
            "idx16": pp["idx16"][c],
            "wcls": pp["wcls"][c],
            "dis2d": pp["dis2d"][c],
            "b1row": b1row,
        }
        for c in cores
    ]
    r2 = run_bass_kernel_spmd(nc2, in2, cores, trace=False)
    table2 = np.ascontiguousarray(
        np.concatenate([r2.results[c]["rpk"] for c in cores], axis=0)
    )

    # NEFF3
    w2cols = np.tile(W2.T.reshape(1, OUT * HID), (P, 1)).astype(np.float32)
    b2row = np.tile(b2[None, :], (P, 1)).astype(np.float32)
    in3 = [
        {
            "table": table2,
            "idx16": pp["idx16"][c],
            "wcls": pp["wcls"][c],
            "dis2d": pp["dis2d"][c],
            "w2cols": w2cols,
            "b2row": b2row,
        }
        for c in cores
    ]
    r3 = run_bass_kernel_spmd(nc3, in3, cores, trace=False)
    kernel._last_inmaps = (in1, in2, in3)
    kernel._last_ncs = (nc1, nc2, nc3)

    out = np.empty((N_NODES, OUT), dtype=np.float32)
    for c in cores:
        ids = ranked[c::N_CORES]
        out[ids] = r3.results[c]["out"][: len(ids)]
    return out
